# revision 1
# baseline (speedup 1.0000x reference)
"""Trainium2 Bass kernel for nn_ConvAttention (sparse_attention).

Reference computes, per batch b and query position i (along L):
    qkv = W1 @ x (1x1 conv);  Q,K,V split
    S[b,i,j] = conv5x5(Q[b,i] + K[b,j]) + b2
    attn     = softmax_j(S)
    out[b,i] = sum_j attn[b,i,j] * V[b,j]

Key algebra (exact): conv is linear, so conv(Q_i+K_j) = conv(Q_i)+conv(K_j);
the Q_i, b2, and conv(b1k) terms are constant along the softmax axis j and
cancel exactly.  attn is therefore independent of i and
    out = sum_j softmax_j(conv5x5(W1k @ x_j)) * (W1v @ x_j + b1v)
with the 1x1 K-projection folded into the conv weights on the host:
    W2eff[o,c,dy,dx] = sum_k W2[o,k,dy,dx] * W1k[k,c].

Sharding: 8 cores = 2 batches x 4 row-quads.  Core m owns batch m//4 and
output rows 4q..4q+3 (q = m%4); it holds input rows 4q-2..4q+5 as four
row-PAIR tiles with partitions = (row parity s, channel c).  This packs the
5x5 conv as dense 128x128 matmuls: contraction (2 rows x 64 c_in), output
(2 out-rows x 64 c_out), 15 matmuls of N=512 per out-row-pair (83% dense vs
the 50% of batch-block-diagonal packing).  All data moves in bf16 (DMA is
~330 GB/s effective; halving bytes halves the load time).

Schedule: input DMAs are chunked in consumption order so the first conv
matmul's semaphore fires just after the 3us PE p-state cliff (matmuls
dispatched later than that run at the full 2.4 GHz); a dummy-matmul
accumulation group keeps PE busy through the DMA lead-in so the p-state
clock never resets.  Row-pair 0 is computed group-major (following the DMA
stream), row-pair 1 bank-major so each score bank finishes as early as
possible and its softmax chain (ACT exp -> mul -> fused DVE reduce, all
bf16 SBUF for the DVE fast modes) overlaps the remaining conv.  exp-sums
and exp*V-sums ship to the host, which does the final divide, adds b1v,
and broadcasts over l (attn is i-independent).
"""

import numpy as np

B, C, H, W, L = 2, 64, 16, 16, 32
NCORES = 8
QH = 4                      # owned output rows per core
NPAIR = 4                   # input row-pair tiles per core (8 halo rows)
WPAD = W + 4                # zero-padded width
P = 2 * C                   # partitions: (row parity, channel)
# Score-bank column chunks (start, len) per row-pair.  rb0's banks finish
# mid-conv (chains fully hidden), so two wide banks minimize op overhead;
# rb1's banks finish near the end, so a train of small banks keeps each
# softmax chain short and the final one tiny.
WCS0 = [(0, 8), (8, 8)]
WCS1 = [(0, 7), (13, 3), (7, 3), (10, 3)]

_PLAN = None


def _np_bf16():
    import ml_dtypes
    return ml_dtypes.bfloat16


class _Plan:
    def __init__(self):
        import concourse.bacc as bacc
        import concourse.tile as tile
        from concourse import mybir

        f32 = mybir.dt.float32
        bf16 = mybir.dt.bfloat16
        nc = bacc.Bacc("TRN2", target_bir_lowering=False, debug=False,
                       num_devices=NCORES)

        xa_d = nc.dram_tensor("xa", [P, 2, W, L], bf16, kind="ExternalInput")
        xb_d = nc.dram_tensor("xb", [P, 2, W, L], bf16, kind="ExternalInput")
        wa_d = nc.dram_tensor("wa", [P, 5, P], bf16, kind="ExternalInput")
        wb_d = nc.dram_tensor("wb", [P, 11, P], bf16, kind="ExternalInput")
        o_d = nc.dram_tensor("o", [P, 2, W], bf16, kind="ExternalOutput")
        o2_d = nc.dram_tensor("o2", [P, 2, W], bf16, kind="ExternalOutput")

        with tile.TileContext(nc) as tc:
            with (
                tc.tile_pool(name="sb", bufs=1) as sb,
                tc.tile_pool(name="psum", bufs=1, space="PSUM") as psum,
            ):
                xq = sb.tile([P, NPAIR, W, L], bf16, tag="xq")
                wt = sb.tile([P, 16, P], bf16, tag="wt")
                wdum = sb.tile([P, 128], bf16, tag="wdum")

                nc.gpsimd.memset(wdum[:], 0)

                # Input DMAs in consumption order.  The first conv group
                # needs (xq pair 0, wa); wb (p1, p2, wv) and xb (pairs 2,3)
                # stream in behind.  x is stored unpadded: edge taps below
                # clamp their output range instead of multiplying pad zeros.
                nc.sync.dma_start(out=xq[:, 0:2], in_=xa_d[:])
                nc.sync.dma_start(out=wt[:, 0:5, :], in_=wa_d[:])
                nc.sync.dma_start(out=wt[:, 5:16, :], in_=wb_d[:])
                nc.sync.dma_start(out=xq[:, 2:4], in_=xb_d[:])

                scores = {(rb, wci): psum.tile([P, n, L], f32,
                                               tag=f"s{rb}{wci}",
                                               name=f"s{rb}{wci}")
                          for rb, wcs in ((0, WCS0), (1, WCS1))
                          for wci, (ws, n) in enumerate(wcs)}
                vps = [psum.tile([P, W, L], f32, tag=f"vp{t}", name=f"vp{t}")
                       for t in range(2)]
                vsb = [sb.tile([P, W, L], bf16, tag=f"vs{t}", name=f"vs{t}")
                       for t in range(2)]
                osum = [sb.tile([P, 2, W], bf16, tag=f"os{rb}",
                        name=f"os{rb}") for rb in range(2)]

                # PE keep-warm: the p-state clock resets if PE idles for
                # multiple microseconds, and a matmul dispatched within 3us
                # of the reset runs at half clock.  A ~2.8us accumulation
                # group of dummy matmuls keeps PE busy through the DMA
                # lead-in so the real conv (dispatched after t>4us) runs at
                # the full 2.4 GHz.  The dummies write the V2 PSUM bank,
                # which is reset (start=True) by the real V projection later.
                # 20 long + 20 tiny dummies: a conv matmul's cost is locked
                # when its slot in the PE queue frees (= when the warmup ~19
                # pairs ahead completes), so the long ones pace that past
                # the 3us cliff while the tiny tail drains well before the
                # first weight DMA lands.
                NWARM = 40
                for k in range(NWARM):
                    wid = 4 if k < 20 else 1
                    nc.tensor.matmul(vps[1][:, 0:wid, :], lhsT=wdum[:],
                                     rhs=wdum[:, 0:32 * wid],
                                     start=(k == 0),
                                     stop=(k == NWARM - 1))

                def taps(ws, n):
                    # Out-column range [a, b) valid for tap dx on unpadded x
                    # (out col w reads in col w+dx-2).  dx=2 is always full
                    # range and goes first so the accumulation group's start
                    # matmul initializes the whole bank.
                    for dx in (2, 3, 4, 0, 1):
                        a = max(ws, 2 - dx)
                        b = min(ws + n, W + 2 - dx)
                        if b > a:
                            yield dx, a, b

                def conv_taps(rb, wci, ws, n, p):
                    tl = list(taps(ws, n))
                    for k, (dx, a, b) in enumerate(tl):
                        nc.tensor.matmul(
                            scores[(rb, wci)][:, a - ws:b - ws, :],
                            lhsT=wt[:, 5 * p + dx, :],
                            rhs=xq[:, rb + p, a + dx - 2:b + dx - 2, :],
                            start=(p == 0 and k == 0),
                            stop=(p == 2 and k == len(tl) - 1),
                        )

                def conv_group(rb, p, wcs):
                    for wci, (ws, n) in enumerate(wcs):
                        conv_taps(rb, wci, ws, n, p)

                def conv_bank(rb, wci, ws, n):
                    # All inputs resident: order taps bank-by-bank so each
                    # bank finishes as early as possible and its softmax
                    # chain overlaps the remaining conv work.
                    for p in range(3):
                        conv_taps(rb, wci, ws, n, p)

                def chain(rb, wci, ws, n, mul_engine):
                    # E and E*V share one tile so a single reduce yields both
                    # sums (host does the final divide).  rb0's muls go to
                    # the otherwise-idle gpsimd to keep DVE (the tail
                    # bottleneck) free for the reduces.
                    ee = sb.tile([P, 2, n, L], bf16, tag=f"e{rb}{wci}",
                                 name=f"e{rb}{wci}")
                    nc.scalar.activation(
                        ee[:, 0], scores[(rb, wci)][:],
                        func=mybir.ActivationFunctionType.Exp)
                    mul_engine.tensor_mul(ee[:, 1], ee[:, 0],
                                          vsb[rb][:, ws:ws + n, :])
                    with nc.allow_low_precision(
                            reason="32-term bf16 sums; rel tol 2e-2"):
                        nc.vector.tensor_reduce(
                            out=osum[rb][:, :, ws:ws + n], in_=ee[:],
                            axis=mybir.AxisListType.X,
                            op=mybir.AluOpType.add)

                conv_group(0, 0, WCS0)
                conv_group(0, 1, WCS0)
                # V projections: V for out rows (2rb, 2rb+1) = pair rb+1.
                nc.tensor.matmul(vps[0][:], lhsT=wt[:, 15, :],
                                 rhs=xq[:, 1], start=True, stop=True)
                nc.scalar.copy(vsb[0][:], vps[0][:])
                nc.tensor.matmul(vps[1][:], lhsT=wt[:, 15, :],
                                 rhs=xq[:, 2], start=True, stop=True)
                conv_group(0, 2, WCS0)
                for wci, (ws, n) in enumerate(WCS0):
                    chain(0, wci, ws, n, nc.gpsimd)
                # V2's SBUF copy sits on ACT after rb0's exps so it doesn't
                # delay them; rb1's first mul needs it ~1.5us later.
                nc.scalar.copy(vsb[1][:], vps[1][:])
                nc.sync.dma_start(out=o_d[:], in_=osum[0][:])
                for wci, (ws, n) in enumerate(WCS1):
                    conv_bank(1, wci, ws, n)
                    chain(1, wci, ws, n, nc.vector)
                nc.sync.dma_start(out=o2_d[:], in_=osum[1][:])

        nc.compile()
        self.nc = nc


def _get_plan():
    global _PLAN
    if _PLAN is None:
        _PLAN = _Plan()
    return _PLAN


def _prep_in_maps(x, W1, W2):
    bf16 = _np_bf16()

    # Fold the K-projection into the conv weights (float64 for accuracy).
    W1k = W1[C:2 * C, :, 0, 0].astype(np.float64)            # [k, c]
    W2eff = np.einsum("okyx,kc->ocyx", W2.astype(np.float64),
                      W1k).astype(np.float32)                # [o, c, 5, 5]
    W1v = W1[2 * C:3 * C, :, 0, 0].astype(np.float32)        # [o, c]

    # Conv lhsT tiles: wt[p*5+dx][(s,ci),(rh,co)] = W2eff[co,ci,2p+s-rh,dx].
    wtiles = np.zeros((15, P, P), np.float32)
    for p in range(3):
        for dx in range(5):
            for s in range(2):
                for rh in range(2):
                    dyi = 2 * p + s - rh
                    if 0 <= dyi <= 4:
                        wtiles[5 * p + dx,
                               64 * s:64 * s + 64,
                               64 * rh:64 * rh + 64] = W2eff[:, :, dyi, dx].T
    # V lhsT: block-diagonal (s,ci)->(s,co) copies of W1v.T.
    wv = np.zeros((P, P), np.float32)
    wv[:C, :C] = W1v.T
    wv[C:, C:] = W1v.T

    wa = np.ascontiguousarray(wtiles[:5].transpose(1, 0, 2)).astype(bf16)
    wb = np.concatenate([wtiles[5:].transpose(1, 0, 2),
                         wv[:, None, :]], axis=1).astype(bf16)

    # x row-pair tiles: [(s,c), t, w, l] = x[bm, c, 4q-2+2t+s, w, l].
    xp = np.zeros((B, C, H + 4, W, L), np.float32)
    xp[:, :, 2:2 + H] = x
    in_maps = []
    for m in range(NCORES):
        bm, q = m // 4, m % 4
        rows = xp[bm, :, 4 * q:4 * q + 8]                    # [c, 8, w, l]
        tiles = rows.reshape(C, NPAIR, 2, W, L).transpose(2, 0, 1, 3, 4)
        tiles = tiles.reshape(P, NPAIR, W, L).astype(bf16)   # [(s,c),t,w,l]
        in_maps.append({
            "xa": np.ascontiguousarray(tiles[:, 0:2]),
            "xb": np.ascontiguousarray(tiles[:, 2:4]),
            "wa": wa, "wb": wb,
        })
    return in_maps


def kernel(x, W1, b1, W2, b2):
    from concourse.bass_utils import run_bass_kernel_spmd

    x = np.asarray(x, dtype=np.float32)
    W1 = np.asarray(W1, dtype=np.float32)
    b1 = np.asarray(b1, dtype=np.float32)
    W2 = np.asarray(W2, dtype=np.float32)

    plan = _get_plan()
    in_maps = _prep_in_maps(x, W1, W2)
    res = run_bass_kernel_spmd(plan.nc, in_maps, core_ids=list(range(NCORES)))

    b1v = b1[2 * C:3 * C].astype(np.float32)
    out = np.empty((B, C, H, W, L), np.float32)
    for m in range(NCORES):
        bm, q = m // 4, m % 4
        o0 = np.asarray(res.results[m]["o"], dtype=np.float32)
        o1 = np.asarray(res.results[m]["o2"], dtype=np.float32)
        o = np.stack([o0.reshape(P, 2, W),
                      o1.reshape(P, 2, W)], axis=1)  # [(rh,c), rb, E/EV, w]
        o = o.reshape(2, C, 2, 2, W)            # [rh, c, rb, E/EV, w]
        val = o[:, :, :, 1] / o[:, :, :, 0]     # [rh, c, rb, w]
        val = val + b1v[None, :, None, None]
        # rows: 4q + 2*rb + rh
        val = val.transpose(1, 2, 0, 3).reshape(C, 4, W)
        out[bm, :, 4 * q:4 * q + 4, :, :] = val[..., None]
    return out



# revision 5
# speedup vs baseline: 1.0131x; 1.0131x over previous
"""Trainium2 Bass kernel for nn_ConvAttention (sparse_attention).

Reference computes, per batch b and query position i (along L):
    qkv = W1 @ x (1x1 conv);  Q,K,V split
    S[b,i,j] = conv5x5(Q[b,i] + K[b,j]) + b2
    attn     = softmax_j(S)
    out[b,i] = sum_j attn[b,i,j] * V[b,j]

Key algebra (exact): conv is linear, so conv(Q_i+K_j) = conv(Q_i)+conv(K_j);
the Q_i, b2, and conv(b1k) terms are constant along the softmax axis j and
cancel exactly.  attn is therefore independent of i and
    out = sum_j softmax_j(conv5x5(W1k @ x_j)) * (W1v @ x_j + b1v)
with the 1x1 K-projection folded into the conv weights on the host:
    W2eff[o,c,dy,dx] = sum_k W2[o,k,dy,dx] * W1k[k,c].

Sharding: 8 cores = 2 batches x 4 row-quads.  Core m owns batch m//4 and
output rows 4q..4q+3 (q = m%4); it holds input rows 4q-2..4q+5 as four
row-PAIR tiles with partitions = (row parity s, channel c).  This packs the
5x5 conv as dense 128x128 matmuls: contraction (2 rows x 64 c_in), output
(2 out-rows x 64 c_out), 15 matmuls of N<=512 per out-row-pair.  All data
moves in bf16 in one SBUF blob, DMA-chunked in consumption order; the
first chunk is just (dx=2 tap, x pair 0) so conv matmuls start ~3.3us,
right after the p-state cliff (PE busy-anchor stays at ~0 because short
engine idles don't reset it; dummy matmuls bridge to the cliff and keep
the cost-lock lag past it).

Tail: each score bank's softmax chain (ACT exp -> DVE mul -> DVE reduce of
E and E*V) overlaps remaining conv.  rb0's sums leave via a plain DMA
mid-kernel; rb1's leave via a gpsimd scatter-add whose descriptors are
pre-generated at ~1us (prepare_only; the osum read dep defers to the
trigger), so the post-reduce path is trigger_dma + transfer + DMA-sem
instead of the 625ns HWDGE + 650ns DGE dma_start chain.  Host does the
final divide, adds b1v, broadcasts over l (attn is i-independent).
"""

import numpy as np

B, C, H, W, L = 2, 64, 16, 16, 32
NCORES = 8
QH = 4                      # owned output rows per core
NPAIR = 4                   # input row-pair tiles per core (8 halo rows)
P = 2 * C                   # partitions: (row parity, channel)
WCS0 = [(0, 8), (8, 8)]
WCS1 = [(0, 7), (13, 3), (7, 3), (10, 3)]
TAPORD = (2, 3, 4, 0, 1)    # dx consumption order (dx=2 is full-width)

# SBUF blob layout (bf16 elems per partition).  Weight tile (p, dx) sits at
# OFF_W[p] + 128*TAPIDX[dx] (consumption order within each p block).
# Four DMA chunks: each extra DMA costs 650ns of serialized HWDGE on every
# later chunk, so finer chunking starves the p1/p2 groups.
TAPIDX = {dx: i for i, dx in enumerate(TAPORD)}
OFF_W0 = {dx: 128 * TAPIDX[dx] for dx in range(5)}  # p=0 taps
OFF_W = {1: 1152, 2: 2304}
OFF_X = {0: 640, 1: 1792, 2: 3072, 3: 3584}
OFF_WV = 2944
BLOB = 4096
# DMA chunks [start, len) in consumption order.
CHUNKS = [(0, 1152), (1152, 1152), (2304, 1280), (3584, 512)]

_PLAN = None


def _np_bf16():
    import ml_dtypes
    return ml_dtypes.bfloat16


class _Plan:
    def __init__(self):
        import concourse.bacc as bacc
        import concourse.tile as tile
        from concourse import bass_types, mybir

        f32 = mybir.dt.float32
        bf16 = mybir.dt.bfloat16
        i16 = mybir.dt.int16
        nc = bacc.Bacc("TRN2", target_bir_lowering=False, debug=False,
                       num_devices=NCORES)

        d_d = [nc.dram_tensor(f"d{k}", [P, ln], bf16, kind="ExternalInput")
               for k, (st, ln) in enumerate(CHUNKS)]
        i_d = nc.dram_tensor("i", [16, 8], mybir.dt.int16,
                             kind="ExternalInput")
        o_d = nc.dram_tensor("o", [P, 2, W], bf16, kind="ExternalOutput")
        # 64 f32 per partition (scatter rows must be 256B); E sums live at
        # [0:W], E*V sums at [2W:3W], rest is pad.
        o2_d = nc.dram_tensor("o2", [P, 4 * W], f32, kind="ExternalOutput")

        with tile.TileContext(nc) as tc:
            with (
                tc.tile_pool(name="sb", bufs=1) as sb,
                tc.tile_pool(name="psum", bufs=1, space="PSUM") as psum,
            ):
                blob = sb.tile([P, BLOB], bf16, tag="blob")
                wdum = sb.tile([P, 128], bf16, tag="wdum")
                scidx = sb.tile([16, 8], i16, tag="scidx")

                nc.gpsimd.memset(wdum[:], 0)

                # Input DMAs in consumption order.
                for k, (st, ln) in enumerate(CHUNKS):
                    nc.sync.dma_start(out=blob[:, st:st + ln], in_=d_d[k][:])
                # Pre-zero o2 (the scatter-add accumulates into it); scatter
                # row indices ride in as a tiny input (hw iota feeding
                # scatter descriptors proved unreliable).
                nc.sync.dma_start(out=o2_d[:],
                                  in_=wdum[:].bitcast(f32))
                nc.sync.dma_start(out=scidx[:], in_=i_d[:])

                def wt(p, dx):
                    off = OFF_W0[dx] if p == 0 else OFF_W[p] + 128 * TAPIDX[dx]
                    return blob[:, off:off + 128]

                def xw(t):
                    off = OFF_X[t]
                    return blob[:, off:off + W * L].rearrange(
                        "p (w l) -> p w l", l=L)

                wv = blob[:, OFF_WV:OFF_WV + 128]

                scores = {(rb, wci): psum.tile([P, n, L], f32,
                                               tag=f"s{rb}{wci}",
                                               name=f"s{rb}{wci}")
                          for rb, wcs in ((0, WCS0), (1, WCS1))
                          for wci, (ws, n) in enumerate(wcs)}
                vps = [psum.tile([P, W, L], f32, tag=f"vp{t}", name=f"vp{t}")
                       for t in range(2)]
                vsb = [sb.tile([P, W, L], bf16, tag=f"vs{t}", name=f"vs{t}")
                       for t in range(2)]
                osum = sb.tile([P, 2, W], bf16, tag="os0", name="os0")
                # [P, (E/EV), (used/pad), W]; zeroed so the pad lanes add 0.
                osum1 = sb.tile([P, 2, 2, W], f32, tag="os1", name="os1")
                nc.gpsimd.memset(osum1[:], 0)

                # PE keep-warm: the p-state cliff sits at busy-anchor+3us
                # (anchor ~0 here).  20 long dummies reach past the cliff so
                # conv matmuls' costs lock at full speed (the cost locks ~38
                # queue slots ahead of execution); the 1-col tail drains in
                # ~30ns so conv isn't blocked behind it.
                NWARM = 40
                for k in range(NWARM):
                    wid = 4 if k < 20 else 1
                    nc.tensor.matmul(vps[1][:, 0:wid, 0:(L if wid == 4 else 1)],
                                     lhsT=wdum[:],
                                     rhs=wdum[:, 0:(128 if wid == 4 else 1)],
                                     start=(k == 0),
                                     stop=(k == NWARM - 1))

                def taps(ws, n):
                    for dx in TAPORD:
                        a = max(ws, 2 - dx)
                        b = min(ws + n, W + 2 - dx)
                        if b > a:
                            yield dx, a, b

                def mm(rb, wci, ws, p, dx, a, b, start, stop):
                    nc.tensor.matmul(
                        scores[(rb, wci)][:, a - ws:b - ws, :],
                        lhsT=wt(p, dx),
                        rhs=xw(rb + p)[:, a + dx - 2:b + dx - 2, :],
                        start=start, stop=stop)

                def conv_taps(rb, wci, ws, n, p):
                    tl = list(taps(ws, n))
                    for k, (dx, a, b) in enumerate(tl):
                        mm(rb, wci, ws, p, dx, a, b,
                           start=(p == 0 and k == 0),
                           stop=(p == 2 and k == len(tl) - 1))

                def conv_group_tapmajor(rb, p, wcs):
                    # tap-major: dx2 of every bank first, then dx3, ... so
                    # the first DMA chunk (dx2 tap + x pair) unblocks work.
                    for ti, dx in enumerate(TAPORD):
                        for wci, (ws, n) in enumerate(wcs):
                            a = max(ws, 2 - dx)
                            b = min(ws + n, W + 2 - dx)
                            if b > a:
                                mm(rb, wci, ws, p, dx, a, b,
                                   start=(p == 0 and ti == 0), stop=False)

                def conv_group(rb, p, wcs):
                    for wci, (ws, n) in enumerate(wcs):
                        conv_taps(rb, wci, ws, n, p)

                def conv_bank(rb, wci, ws, n):
                    for p in range(3):
                        conv_taps(rb, wci, ws, n, p)

                def chain(rb, wci, ws, n, last=False, red=None):
                    # E and E*V share one tile so a single reduce yields both
                    # sums (host does the final divide).
                    red = red or nc.vector
                    ee = sb.tile([P, 2, n, L], bf16, tag=f"e{rb}{wci}",
                                 name=f"e{rb}{wci}")
                    nc.scalar.activation(
                        ee[:, 0], scores[(rb, wci)][:],
                        func=mybir.ActivationFunctionType.Exp)
                    if last:
                        # Split reduces: E right after exp, EV after the mul,
                        # so the post-conv critical chain is exp+mul+half-
                        # reduce.
                        with nc.allow_low_precision(reason="rel tol 2e-2"):
                            nc.vector.tensor_reduce(
                                out=osum1[:, 0, 0, ws:ws + n],
                                in_=ee[:, 0],
                                axis=mybir.AxisListType.X,
                                op=mybir.AluOpType.add)
                        nc.vector.tensor_mul(ee[:, 1], ee[:, 0],
                                             vsb[rb][:, ws:ws + n, :])
                        with nc.allow_low_precision(reason="rel tol 2e-2"):
                            nc.vector.tensor_reduce(
                                out=osum1[:, 1, 0, ws:ws + n],
                                in_=ee[:, 1],
                                axis=mybir.AxisListType.X,
                                op=mybir.AluOpType.add)
                        return
                    nc.vector.tensor_mul(ee[:, 1], ee[:, 0],
                                         vsb[rb][:, ws:ws + n, :])
                    out_ap = (osum[:, :, ws:ws + n] if rb == 0
                              else osum1[:, :, 0, ws:ws + n])
                    with nc.allow_low_precision(
                            reason="32-term bf16 sums; rel tol 2e-2"):
                        red.tensor_reduce(
                            out=out_ap, in_=ee[:],
                            axis=mybir.AxisListType.X,
                            op=mybir.AluOpType.add)

                conv_group_tapmajor(0, 0, WCS0)
                conv_group(0, 1, WCS0)
                # V projections: V for out rows (2rb, 2rb+1) = pair rb+1.
                nc.tensor.matmul(vps[0][:], lhsT=wv,
                                 rhs=xw(1), start=True, stop=True)
                nc.scalar.copy(vsb[0][:], vps[0][:])
                nc.tensor.matmul(vps[1][:], lhsT=wv,
                                 rhs=xw(2), start=True, stop=True)
                conv_group(0, 2, WCS0)
                for wci, (ws, n) in enumerate(WCS0):
                    chain(0, wci, ws, n)
                # V2's SBUF copy sits on ACT after rb0's exps so it doesn't
                # delay them; rb1's first mul needs it ~1.5us later.
                nc.scalar.copy(vsb[1][:], vps[1][:])
                nc.sync.dma_start(out=o_d[:], in_=osum[:])
                nwcs1 = len(WCS1)
                for wci, (ws, n) in enumerate(WCS1):
                    conv_bank(1, wci, ws, n)
                    chain(1, wci, ws, n)
                # rb1 output: plain DMA (the SWDGE prepare/trigger path
                # reads stale data on the execution backend, so the
                # 625+650ns issue chain after the last reduce is the best
                # available here).
                nc.sync.dma_start(out=o2_d[:],
                                  in_=osum1[:].rearrange("p a u w -> p (a u w)"))

        nc.compile()
        self.nc = nc


def _get_plan():
    global _PLAN
    if _PLAN is None:
        _PLAN = _Plan()
    return _PLAN


def _prep_in_maps(x, W1, W2):
    bf16 = _np_bf16()

    # Fold the K-projection into the conv weights (float64 for accuracy).
    W1k = W1[C:2 * C, :, 0, 0].astype(np.float64)            # [k, c]
    W2eff = np.einsum("okyx,kc->ocyx", W2.astype(np.float64),
                      W1k).astype(np.float32)                # [o, c, 5, 5]
    W1v = W1[2 * C:3 * C, :, 0, 0].astype(np.float32)        # [o, c]

    # Conv lhsT tiles: wtiles[p*5+dx][(s,ci),(rh,co)] = W2eff[co,ci,2p+s-rh,dx]
    wtiles = np.zeros((3, 5, P, P), np.float32)
    for p in range(3):
        for dx in range(5):
            for s in range(2):
                for rh in range(2):
                    dyi = 2 * p + s - rh
                    if 0 <= dyi <= 4:
                        wtiles[p, dx,
                               64 * s:64 * s + 64,
                               64 * rh:64 * rh + 64] = W2eff[:, :, dyi, dx].T
    # V lhsT: block-diagonal (s,ci)->(s,co) copies of W1v.T.
    wv = np.zeros((P, P), np.float32)
    wv[:C, :C] = W1v.T
    wv[C:, C:] = W1v.T

    # x row-pair tiles: [(s,c), t, w, l] = x[bm, c, 4q-2+2t+s, w, l].
    xp = np.zeros((B, C, H + 4, W, L), np.float32)
    xp[:, :, 2:2 + H] = x
    in_maps = []
    for m in range(NCORES):
        bm, q = m // 4, m % 4
        rows = xp[bm, :, 4 * q:4 * q + 8]                    # [c, 8, w, l]
        tiles = rows.reshape(C, NPAIR, 2, W, L).transpose(2, 0, 1, 3, 4)
        tiles = tiles.reshape(P, NPAIR, W * L)               # [(s,c),t,wl]

        blob = np.zeros((P, BLOB), np.float32)
        for dx in range(5):
            blob[:, OFF_W0[dx]:OFF_W0[dx] + 128] = wtiles[0, dx]
        for p in (1, 2):
            for dx in range(5):
                off = OFF_W[p] + 128 * TAPIDX[dx]
                blob[:, off:off + 128] = wtiles[p, dx]
        blob[:, OFF_WV:OFF_WV + 128] = wv
        for t in range(NPAIR):
            blob[:, OFF_X[t]:OFF_X[t] + W * L] = tiles[:, t]
        blob = blob.astype(bf16)
        im = {f"d{k}": np.ascontiguousarray(blob[:, st:st + ln])
              for k, (st, ln) in enumerate(CHUNKS)}
        im["i"] = np.ascontiguousarray(
            np.arange(P, dtype=np.int16).reshape(8, 16).T)
        in_maps.append(im)
    return in_maps


def kernel(x, W1, b1, W2, b2):
    from concourse.bass_utils import run_bass_kernel_spmd

    x = np.asarray(x, dtype=np.float32)
    W1 = np.asarray(W1, dtype=np.float32)
    b1 = np.asarray(b1, dtype=np.float32)
    W2 = np.asarray(W2, dtype=np.float32)

    plan = _get_plan()
    in_maps = _prep_in_maps(x, W1, W2)
    res = run_bass_kernel_spmd(plan.nc, in_maps, core_ids=list(range(NCORES)))

    b1v = b1[2 * C:3 * C].astype(np.float32)
    out = np.empty((B, C, H, W, L), np.float32)
    for m in range(NCORES):
        bm, q = m // 4, m % 4
        o0 = np.asarray(res.results[m]["o"], dtype=np.float32)
        o1 = np.asarray(res.results[m]["o2"],
                        dtype=np.float32).reshape(P, 2, 2, W)[:, :, 0]
        o = np.stack([o0.reshape(P, 2, W), o1],
                     axis=1)                  # [(rh,c), rb, E/EV, w]
        o = o.reshape(2, C, 2, 2, W)            # [rh, c, rb, E/EV, w]
        val = o[:, :, :, 1] / o[:, :, :, 0]     # [rh, c, rb, w]
        val = val + b1v[None, :, None, None]
        # rows: 4q + 2*rb + rh
        val = val.transpose(1, 2, 0, 3).reshape(C, 4, W)
        out[bm, :, 4 * q:4 * q + 4, :, :] = val[..., None]
    return out


# revision 8
# speedup vs baseline: 1.0637x; 1.0500x over previous
"""Trainium2 Bass kernel for nn_ConvAttention (sparse_attention).

Same algebra as the bf16 version (softmax is query-independent; the conv
weights fold the K-projection), but the score conv runs on the PE in fp8
DoubleRow mode: each matmul contracts TWO 128-deep k-tiles at 0.5
cycles/output-column, i.e. 4x the bf16 MAC rate.

Precision: e4m3 alone costs ~3e-2 relative error (gate is 2e-2), so the
conv is 3-term compensated exactly:
    W (x) x ~= Whi (x) xhi + Whi (x) xlo + Wlo (x) xhi
with Whi = fp8(W*SW), Wlo = fp8(W*SW - Whi), xhi = fp8(x*SX),
xlo = fp8(x*SX - xhi).  All terms share one PSUM accumulation at scale
SW*SX; the dropped Wlo(x)xlo term is ~1e-3 relative.  The cross terms
pair into single DoubleRow matmuls ((Whi,xlo)+(Wlo,xhi)), mains pair
across the p row-pair tiles, so a bank costs 23 DR matmuls vs 15 bf16
matmuls of twice the cycles: conv drops from ~5.9us to ~4.6us of PE.
The exp descale (1/(SW*SX)) folds into the ACT exp's scale input; the V
projection is also 3-term fp8 (scale folded into the V PSUM->SBUF copy).

Everything else (row-pair sharding, one-blob chunked DMA, dummy-matmul
p-state warmup, per-bank exp->mul->reduce chains, host-side divide) is
as in the bf16 kernel.
"""

import numpy as np

B, C, H, W, L = 2, 64, 16, 16, 32
NCORES = 8
NPAIR = 4
P = 2 * C
WCS0 = [(0, 8), (8, 8)]
WCS1 = [(0, 7), (13, 3), (7, 3), (10, 3)]
TAPORD = (2, 3, 4, 0, 1)
TAPIDX = {dx: i for i, dx in enumerate(TAPORD)}

SW = 32.0                   # conv-weight fp8 scale
SX = 8.0                    # x fp8 scale
SWV = 16.0                  # V-weight fp8 scale

# fp8 blob layout (1-byte elems per partition).  Taps store [hi|lo] pairs
# of 128x128 tiles; x pairs store [lo(512)|hi(512)] planes.
WT = {0: 0, 1: 2432, 2: 4736}
XP = {0: 1280, 1: 3712, 2: 6272, 3: 7296}
ZOFF = 2304                 # 128 zeros (DoubleRow padding partner)
WVOFF = 6016                # [wv_hi | wv_lo]
BLOB = 8320
CHUNKS = [(0, 2432), (2432, 2304), (4736, 2560), (7296, 1024)]

_PLAN = None


def _fp8():
    import ml_dtypes
    return ml_dtypes.float8_e4m3


def _np_bf16():
    import ml_dtypes
    return ml_dtypes.bfloat16


class _Plan:
    def __init__(self):
        import concourse.bacc as bacc
        import concourse.tile as tile
        from concourse import bass_types, mybir

        f32 = mybir.dt.float32
        bf16 = mybir.dt.bfloat16
        fp8 = mybir.dt.float8e4
        DR = mybir.MatmulPerfMode.DoubleRow
        nc = bacc.Bacc("TRN2", target_bir_lowering=False, debug=False,
                       num_devices=NCORES)

        d_d = [nc.dram_tensor(f"d{k}", [P, ln], fp8, kind="ExternalInput")
               for k, (st, ln) in enumerate(CHUNKS)]
        o_d = nc.dram_tensor("o", [P, 2, W], bf16, kind="ExternalOutput")
        o2_d = nc.dram_tensor("o2", [P, 2, W], bf16,
                              kind="ExternalOutput")

        with tile.TileContext(nc) as tc:
            with (
                tc.tile_pool(name="sb", bufs=1) as sb,
                tc.tile_pool(name="psum", bufs=1, space="PSUM") as psum,
            ):
                blob = sb.tile([P, BLOB], fp8, tag="blob")
                wdum = sb.tile([P, 128], bf16, tag="wdum")
                nc.gpsimd.memset(wdum[:], 0)

                for k, (st, ln) in enumerate(CHUNKS):
                    nc.sync.dma_start(out=blob[:, st:st + ln], in_=d_d[k][:])

                bt = blob[:]

                def wap(off, d):
                    # lhsT [K=128, 2, 128]: DoubleRow weight tile pair.
                    return bass_types.AP(bt.tensor, bt.offset + off,
                                         [[BLOB, P], [d, 2], [1, 128]])

                def xap(off, d, ncols):
                    # rhs [K=128, 2, ncols, L]: DoubleRow moving tile pair.
                    return bass_types.AP(bt.tensor, bt.offset + off,
                                         [[BLOB, P], [d, 2], [L, ncols],
                                          [1, L]])

                def sap(off, ncols):
                    # single-tile rhs [K, ncols, L] (sliver matmuls).
                    return bass_types.AP(bt.tensor, bt.offset + off,
                                         [[BLOB, P], [L, ncols], [1, L]])

                def wsap(off):
                    return bass_types.AP(bt.tensor, bt.offset + off,
                                         [[BLOB, P], [1, 128]])

                def wtoff(p, dx):
                    return WT[p] + 256 * TAPIDX[dx]

                scores = {(rb, wci): psum.tile([P, n, L], f32,
                                               tag=f"s{rb}{wci}",
                                               name=f"s{rb}{wci}")
                          for rb, wcs in ((0, WCS0), (1, WCS1))
                          for wci, (ws, n) in enumerate(wcs)}
                vps = [psum.tile([P, W, L], f32, tag=f"vp{t}", name=f"vp{t}")
                       for t in range(2)]
                vsb = [sb.tile([P, W, L], bf16, tag=f"vs{t}", name=f"vs{t}")
                       for t in range(2)]
                osum = sb.tile([P, 2, W], bf16, tag="os0", name="os0")
                osum1 = sb.tile([P, 2, W], bf16, tag="os1", name="os1")

                NWARM = 40
                for k in range(NWARM):
                    wid = 4 if k < 20 else 1
                    nc.tensor.matmul(
                        vps[1][:, 0:wid, 0:(L if wid == 4 else 1)],
                        lhsT=wdum[:], rhs=wdum[:, 0:(128 if wid == 4 else 1)],
                        start=(k == 0), stop=(k == NWARM - 1))

                def clip(ws, n, dx):
                    return max(ws, 2 - dx), min(ws + n, W + 2 - dx)

                def drmm(rb, wci, ws, woff, wd, xoff, xd, a, b,
                         start=False, stop=False):
                    nc.tensor.matmul(
                        scores[(rb, wci)][:, a - ws:b - ws, :],
                        lhsT=wap(woff, wd),
                        rhs=xap(xoff + a * L, xd, b - a),
                        start=start, stop=stop, perf_mode=DR)

                def smm(rb, wci, ws, woff, xoff, a, b, stop=False):
                    nc.tensor.matmul(
                        scores[(rb, wci)][:, a - ws:b - ws, :],
                        lhsT=wsap(woff),
                        rhs=sap(xoff + a * L, b - a),
                        start=False, stop=stop)

                # Per (bank, p) emission so the schedule can follow the DMA
                # chunk stream.  p pass 0/1/2: cross terms for that p; the
                # p1 pass also does the (p0,p1) main pair; the p2 pass does
                # the p2 mains (zero-padded dx2 + cross-dx pairs + slivers).
                def conv_pass(rb, wci, ws, n, p):
                    xb = [XP[rb + q] for q in range(3)]
                    first = True
                    for dx in TAPORD:
                        a, b = clip(ws, n, dx)
                        if b <= a:
                            continue
                        sh = (dx - 2) * L
                        if p < 2:
                            # cross: Whi_p (x) xlo_p + Wlo_p (x) xhi_p
                            drmm(rb, wci, ws, wtoff(p, dx), 128,
                                 xb[p] + sh, 512, a, b,
                                 start=(p == 0 and first))
                            first = False
                            if p == 1:
                                # mains (p0, p1) on the hi planes
                                drmm(rb, wci, ws, wtoff(0, dx),
                                     wtoff(1, dx) - wtoff(0, dx),
                                     xb[0] + 512 + sh, xb[1] - xb[0], a, b)
                        else:
                            drmm(rb, wci, ws, wtoff(2, dx), 128,
                                 xb[2] + sh, 512, a, b)
                    if p == 2:
                        # p2 mains.  dx=2 (full range) pairs with the zero
                        # tile; (3,4) and (0,1) pair cross-dx on their range
                        # intersections, with sliver singles for the rest.
                        a, b = ws, ws + n
                        drmm(rb, wci, ws, ZOFF, wtoff(2, 2) - ZOFF,
                             xb[2], 512, a, b)
                        slv = []
                        for dxa, dxb in ((3, 4), (0, 1)):
                            aa, ba = clip(ws, n, dxa)
                            ab, bb = clip(ws, n, dxb)
                            ia, ib = max(aa, ab), min(ba, bb)
                            if ib > ia:
                                drmm(rb, wci, ws, wtoff(2, dxa),
                                     256 * (TAPIDX[dxb] - TAPIDX[dxa]),
                                     xb[2] + 512 + (dxa - 2) * L,
                                     (dxb - dxa) * L, ia, ib)
                            for dx, (c, d) in ((dxa, (aa, ba)),
                                               (dxb, (ab, bb))):
                                for sa, sb_ in ((c, min(d, ia)),
                                                (max(c, ib), d)):
                                    if sb_ > sa:
                                        slv.append((dx, sa, sb_))
                        for k, (dx, sa, sb_) in enumerate(slv):
                            smm(rb, wci, ws, wtoff(2, dx),
                                xb[2] + 512 + (dx - 2) * L, sa, sb_,
                                stop=(k == len(slv) - 1))
                        if not slv:
                            # close the accumulation group with a 1-col
                            # zero-weight single matmul (adds 0).
                            smm(rb, wci, ws, ZOFF, xb[2] + 512, ws, ws + 1,
                                stop=True)

                def conv_group(rb, p, wcs):
                    for wci, (ws, n) in enumerate(wcs):
                        conv_pass(rb, wci, ws, n, p)

                def conv_bank(rb, wci, ws, n):
                    for p in range(3):
                        conv_pass(rb, wci, ws, n, p)

                def vproj(rb):
                    # vps[rb] = wv (x) x_{rb+1}, 3-term fp8.
                    xo = XP[rb + 1]
                    nc.tensor.matmul(
                        vps[rb][:], lhsT=wap(WVOFF, 128),
                        rhs=xap(xo, 512, W), start=True, stop=False,
                        perf_mode=DR)
                    nc.tensor.matmul(
                        vps[rb][:], lhsT=wap(ZOFF, WVOFF - ZOFF),
                        rhs=xap(xo, 512, W), start=False, stop=True,
                        perf_mode=DR)

                def chain(rb, wci, ws, n, mule=None):
                    mule = mule or nc.vector
                    ee = sb.tile([P, 2, n, L], bf16, tag=f"e{rb}{wci}",
                                 name=f"e{rb}{wci}")
                    nc.scalar.activation(
                        ee[:, 0], scores[(rb, wci)][:],
                        func=mybir.ActivationFunctionType.Exp,
                        scale=1.0 / (SW * SX))
                    mule.tensor_mul(ee[:, 1], ee[:, 0],
                                    vsb[rb][:, ws:ws + n, :])
                    out_ap = (osum if rb == 0
                              else osum1)[:, :, ws:ws + n]
                    with nc.allow_low_precision(
                            reason="32-term bf16 sums; rel tol 2e-2"):
                        nc.vector.tensor_reduce(
                            out=out_ap, in_=ee[:],
                            axis=mybir.AxisListType.X,
                            op=mybir.AluOpType.add)

                conv_group(0, 0, WCS0)
                conv_group(0, 1, WCS0)
                vproj(0)
                nc.scalar.activation(vsb[0][:], vps[0][:],
                                     func=mybir.ActivationFunctionType.Copy,
                                     scale=1.0 / (SWV * SX))
                vproj(1)
                # bank-major p2 passes so bank0's scores (and its softmax
                # chain, which seeds the serial DVE tail) finish as soon as
                # chunk 3 lands; rb0 muls ride the otherwise-idle gpsimd.
                conv_pass(0, 0, WCS0[0][0], WCS0[0][1], 2)
                chain(0, 0, WCS0[0][0], WCS0[0][1])
                conv_pass(0, 1, WCS0[1][0], WCS0[1][1], 2)
                chain(0, 1, WCS0[1][0], WCS0[1][1])
                nc.scalar.activation(vsb[1][:], vps[1][:],
                                     func=mybir.ActivationFunctionType.Copy,
                                     scale=1.0 / (SWV * SX))
                nc.sync.dma_start(out=o_d[:], in_=osum[:])
                for wci, (ws, n) in enumerate(WCS1):
                    conv_bank(1, wci, ws, n)
                    chain(1, wci, ws, n)
                nc.sync.dma_start(
                    out=o2_d[:],
                    in_=osum1[:])

        nc.compile()
        self.nc = nc


def _get_plan():
    global _PLAN
    if _PLAN is None:
        _PLAN = _Plan()
    return _PLAN


def _prep_in_maps(x, W1, W2):
    fp8 = _fp8()

    W1k = W1[C:2 * C, :, 0, 0].astype(np.float64)
    W2eff = np.einsum("okyx,kc->ocyx", W2.astype(np.float64),
                      W1k).astype(np.float32)
    W1v = W1[2 * C:3 * C, :, 0, 0].astype(np.float32)

    wtiles = np.zeros((3, 5, P, P), np.float32)
    for p in range(3):
        for dx in range(5):
            for s in range(2):
                for rh in range(2):
                    dyi = 2 * p + s - rh
                    if 0 <= dyi <= 4:
                        wtiles[p, dx,
                               64 * s:64 * s + 64,
                               64 * rh:64 * rh + 64] = W2eff[:, :, dyi, dx].T
    wv = np.zeros((P, P), np.float32)
    wv[:C, :C] = W1v.T
    wv[C:, C:] = W1v.T

    def hilo(a, scale):
        hi = (a * scale).astype(fp8)
        lo = (a * scale - hi.astype(np.float32)).astype(fp8)
        return hi, lo

    whi, wlo = hilo(wtiles, SW)          # [3,5,P,P]
    wvhi, wvlo = hilo(wv, SWV)           # [P,P]

    xp = np.zeros((B, C, H + 4, W, L), np.float32)
    xp[:, :, 2:2 + H] = x
    in_maps = []
    for m in range(NCORES):
        bm, q = m // 4, m % 4
        rows = xp[bm, :, 4 * q:4 * q + 8]
        tiles = rows.reshape(C, NPAIR, 2, W, L).transpose(2, 0, 1, 3, 4)
        tiles = tiles.reshape(P, NPAIR, W * L)
        thi, tlo = hilo(tiles, SX)

        blob = np.zeros((P, BLOB), np.float32)
        for p in range(3):
            for dx in range(5):
                off = WT[p] + 256 * TAPIDX[dx]
                blob[:, off:off + 128] = whi[p, dx]
                blob[:, off + 128:off + 256] = wlo[p, dx]
        blob[:, WVOFF:WVOFF + 128] = wvhi
        blob[:, WVOFF + 128:WVOFF + 256] = wvlo
        bq = blob.astype(fp8)
        for t in range(NPAIR):
            bq[:, XP[t]:XP[t] + W * L] = tlo[:, t]
            bq[:, XP[t] + W * L:XP[t] + 2 * W * L] = thi[:, t]
        im = {f"d{k}": np.ascontiguousarray(bq[:, st:st + ln])
              for k, (st, ln) in enumerate(CHUNKS)}
        in_maps.append(im)
    return in_maps


def kernel(x, W1, b1, W2, b2):
    from concourse.bass_utils import run_bass_kernel_spmd

    x = np.asarray(x, dtype=np.float32)
    W1 = np.asarray(W1, dtype=np.float32)
    b1 = np.asarray(b1, dtype=np.float32)
    W2 = np.asarray(W2, dtype=np.float32)

    plan = _get_plan()
    in_maps = _prep_in_maps(x, W1, W2)
    res = run_bass_kernel_spmd(plan.nc, in_maps, core_ids=list(range(NCORES)))

    b1v = b1[2 * C:3 * C].astype(np.float32)
    out = np.empty((B, C, H, W, L), np.float32)
    for m in range(NCORES):
        bm, q = m // 4, m % 4
        o0 = np.asarray(res.results[m]["o"], dtype=np.float32)
        o1 = np.asarray(res.results[m]["o2"],
                        dtype=np.float32).reshape(P, 2, W)
        o = np.stack([o0.reshape(P, 2, W), o1], axis=1)
        o = o.reshape(2, C, 2, 2, W)
        val = o[:, :, :, 1] / o[:, :, :, 0]
        val = val + b1v[None, :, None, None]
        val = val.transpose(1, 2, 0, 3).reshape(C, 4, W)
        out[bm, :, 4 * q:4 * q + 4, :, :] = val[..., None]
    return out


# revision 9
# speedup vs baseline: 1.0816x; 1.0168x over previous
"""Trainium2 Bass kernel for nn_ConvAttention (sparse_attention).

Same algebra as the bf16 version (softmax is query-independent; the conv
weights fold the K-projection), but the score conv runs on the PE in fp8
DoubleRow mode: each matmul contracts TWO 128-deep k-tiles at 0.5
cycles/output-column, i.e. 4x the bf16 MAC rate.

Precision: e4m3 alone costs ~3e-2 relative error (gate is 2e-2), so the
conv is 3-term compensated exactly:
    W (x) x ~= Whi (x) xhi + Whi (x) xlo + Wlo (x) xhi
with Whi = fp8(W*SW), Wlo = fp8(W*SW - Whi), xhi = fp8(x*SX),
xlo = fp8(x*SX - xhi).  All terms share one PSUM accumulation at scale
SW*SX; the dropped Wlo(x)xlo term is ~1e-3 relative.  The cross terms
pair into single DoubleRow matmuls ((Whi,xlo)+(Wlo,xhi)), mains pair
across the p row-pair tiles, so a bank costs 23 DR matmuls vs 15 bf16
matmuls of twice the cycles: conv drops from ~5.9us to ~4.6us of PE.
The exp descale (1/(SW*SX)) folds into the ACT exp's scale input; the V
projection is also 3-term fp8 (scale folded into the V PSUM->SBUF copy).

Everything else (row-pair sharding, one-blob chunked DMA, dummy-matmul
p-state warmup, per-bank exp->mul->reduce chains, host-side divide) is
as in the bf16 kernel.
"""

import numpy as np

B, C, H, W, L = 2, 64, 16, 16, 32
NCORES = 8
NPAIR = 4
P = 2 * C
WCS0 = [(0, 4), (4, 6), (10, 6)]
WCS1 = [(0, 7), (13, 3), (7, 3), (10, 3)]
TAPORD = (2, 3, 4, 0, 1)
TAPIDX = {dx: i for i, dx in enumerate(TAPORD)}

SW = 32.0                   # conv-weight fp8 scale
SX = 8.0                    # x fp8 scale
SWV = 16.0                  # V-weight fp8 scale

# fp8 blob layout (1-byte elems per partition).  Taps store [hi|lo] pairs
# of 128x128 tiles; x pairs store [lo(512)|hi(512)] planes.
WT = {0: 0, 1: 2432, 2: 4736}
XP = {0: 1280, 1: 3712, 2: 6272, 3: 7296}
ZOFF = 2304                 # 128 zeros (DoubleRow padding partner)
WVOFF = 6016                # [wv_hi | wv_lo]
BLOB = 8320
CHUNKS = [(0, 2432), (2432, 2304), (4736, 2560), (7296, 1024)]

_PLAN = None


def _fp8():
    import ml_dtypes
    return ml_dtypes.float8_e4m3


def _np_bf16():
    import ml_dtypes
    return ml_dtypes.bfloat16


class _Plan:
    def __init__(self):
        import concourse.bacc as bacc
        import concourse.tile as tile
        from concourse import bass_types, mybir

        f32 = mybir.dt.float32
        bf16 = mybir.dt.bfloat16
        fp8 = mybir.dt.float8e4
        DR = mybir.MatmulPerfMode.DoubleRow
        nc = bacc.Bacc("TRN2", target_bir_lowering=False, debug=False,
                       num_devices=NCORES)

        d_d = [nc.dram_tensor(f"d{k}", [P, ln], fp8, kind="ExternalInput")
               for k, (st, ln) in enumerate(CHUNKS)]
        o_d = nc.dram_tensor("o", [P, 2, W], bf16, kind="ExternalOutput")
        o2_d = nc.dram_tensor("o2", [P, 2, W], bf16,
                              kind="ExternalOutput")

        with tile.TileContext(nc) as tc:
            with (
                tc.tile_pool(name="sb", bufs=1) as sb,
                tc.tile_pool(name="psum", bufs=1, space="PSUM") as psum,
            ):
                blob = sb.tile([P, BLOB], fp8, tag="blob")
                wdum = sb.tile([P, 128], bf16, tag="wdum")
                nc.gpsimd.memset(wdum[:], 0)

                for k, (st, ln) in enumerate(CHUNKS):
                    nc.sync.dma_start(out=blob[:, st:st + ln], in_=d_d[k][:])

                bt = blob[:]

                def wap(off, d):
                    # lhsT [K=128, 2, 128]: DoubleRow weight tile pair.
                    return bass_types.AP(bt.tensor, bt.offset + off,
                                         [[BLOB, P], [d, 2], [1, 128]])

                def xap(off, d, ncols):
                    # rhs [K=128, 2, ncols, L]: DoubleRow moving tile pair.
                    return bass_types.AP(bt.tensor, bt.offset + off,
                                         [[BLOB, P], [d, 2], [L, ncols],
                                          [1, L]])

                def sap(off, ncols):
                    # single-tile rhs [K, ncols, L] (sliver matmuls).
                    return bass_types.AP(bt.tensor, bt.offset + off,
                                         [[BLOB, P], [L, ncols], [1, L]])

                def wsap(off):
                    return bass_types.AP(bt.tensor, bt.offset + off,
                                         [[BLOB, P], [1, 128]])

                def wtoff(p, dx):
                    return WT[p] + 256 * TAPIDX[dx]

                # PSUM tiles are bank-granular (8 banks); rb1's last
                # bank reuses rb0-bank0's bank (their live ranges are
                # disjoint: the rb0-b0 exp reads it ~2.5us before rb1's
                # last conv bank starts accumulating).
                scores = {(rb, wci): psum.tile([P, n, L], f32,
                                               tag=f"s{rb}{wci}",
                                               name=f"s{rb}{wci}")
                          for rb, wcs in ((0, WCS0), (1, WCS1[:-1]))
                          for wci, (ws, n) in enumerate(wcs)}
                lwci = len(WCS1) - 1
                scores[(1, lwci)] = scores[(0, 0)][:, 0:WCS1[lwci][1], :]
                vps = [psum.tile([P, W, L], f32, tag=f"vp{t}", name=f"vp{t}")
                       for t in range(2)]
                vsb = [sb.tile([P, W, L], bf16, tag=f"vs{t}", name=f"vs{t}")
                       for t in range(2)]
                osum = sb.tile([P, 2, W], bf16, tag="os0", name="os0")
                osum1 = sb.tile([P, 2, W], bf16, tag="os1", name="os1")

                NWARM = 40
                for k in range(NWARM):
                    wid = 4 if k < 20 else 1
                    nc.tensor.matmul(
                        vps[1][:, 0:wid, 0:(L if wid == 4 else 1)],
                        lhsT=wdum[:], rhs=wdum[:, 0:(128 if wid == 4 else 1)],
                        start=(k == 0), stop=(k == NWARM - 1))

                def clip(ws, n, dx):
                    return max(ws, 2 - dx), min(ws + n, W + 2 - dx)

                def drmm(rb, wci, ws, woff, wd, xoff, xd, a, b,
                         start=False, stop=False):
                    nc.tensor.matmul(
                        scores[(rb, wci)][:, a - ws:b - ws, :]
                        if hasattr(scores[(rb, wci)], 'tensor')
                        else scores[(rb, wci)][:, a - ws:b - ws, :],
                        lhsT=wap(woff, wd),
                        rhs=xap(xoff + a * L, xd, b - a),
                        start=start, stop=stop, perf_mode=DR)

                def smm(rb, wci, ws, woff, xoff, a, b, stop=False):
                    nc.tensor.matmul(
                        scores[(rb, wci)][:, a - ws:b - ws, :],
                        lhsT=wsap(woff),
                        rhs=sap(xoff + a * L, b - a),
                        start=False, stop=stop)

                # Per (bank, p) emission so the schedule can follow the DMA
                # chunk stream.  p pass 0/1/2: cross terms for that p; the
                # p1 pass also does the (p0,p1) main pair; the p2 pass does
                # the p2 mains (zero-padded dx2 + cross-dx pairs + slivers).
                def conv_pass(rb, wci, ws, n, p):
                    xb = [XP[rb + q] for q in range(3)]
                    first = True
                    for dx in TAPORD:
                        a, b = clip(ws, n, dx)
                        if b <= a:
                            continue
                        sh = (dx - 2) * L
                        if p < 2:
                            # cross: Whi_p (x) xlo_p + Wlo_p (x) xhi_p
                            drmm(rb, wci, ws, wtoff(p, dx), 128,
                                 xb[p] + sh, 512, a, b,
                                 start=(p == 0 and first))
                            first = False
                            if p == 1:
                                # mains (p0, p1) on the hi planes
                                drmm(rb, wci, ws, wtoff(0, dx),
                                     wtoff(1, dx) - wtoff(0, dx),
                                     xb[0] + 512 + sh, xb[1] - xb[0], a, b)
                        else:
                            drmm(rb, wci, ws, wtoff(2, dx), 128,
                                 xb[2] + sh, 512, a, b)
                    if p == 2:
                        # p2 mains.  dx=2 (full range) pairs with the zero
                        # tile; (3,4) and (0,1) pair cross-dx on their range
                        # intersections, with sliver singles for the rest.
                        a, b = ws, ws + n
                        drmm(rb, wci, ws, ZOFF, wtoff(2, 2) - ZOFF,
                             xb[2], 512, a, b)
                        slv = []
                        for dxa, dxb in ((3, 4), (0, 1)):
                            aa, ba = clip(ws, n, dxa)
                            ab, bb = clip(ws, n, dxb)
                            ia, ib = max(aa, ab), min(ba, bb)
                            if ib > ia:
                                drmm(rb, wci, ws, wtoff(2, dxa),
                                     256 * (TAPIDX[dxb] - TAPIDX[dxa]),
                                     xb[2] + 512 + (dxa - 2) * L,
                                     (dxb - dxa) * L, ia, ib)
                            for dx, (c, d) in ((dxa, (aa, ba)),
                                               (dxb, (ab, bb))):
                                for sa, sb_ in ((c, min(d, ia)),
                                                (max(c, ib), d)):
                                    if sb_ > sa:
                                        slv.append((dx, sa, sb_))
                        for k, (dx, sa, sb_) in enumerate(slv):
                            smm(rb, wci, ws, wtoff(2, dx),
                                xb[2] + 512 + (dx - 2) * L, sa, sb_,
                                stop=(k == len(slv) - 1))
                        if not slv:
                            # close the accumulation group with a 1-col
                            # zero-weight single matmul (adds 0).
                            smm(rb, wci, ws, ZOFF, xb[2] + 512, ws, ws + 1,
                                stop=True)

                def conv_group(rb, p, wcs):
                    for wci, (ws, n) in enumerate(wcs):
                        conv_pass(rb, wci, ws, n, p)

                def conv_bank(rb, wci, ws, n):
                    for p in range(3):
                        conv_pass(rb, wci, ws, n, p)

                def vproj(rb):
                    # vps[rb] = wv (x) x_{rb+1}, 3-term fp8.
                    xo = XP[rb + 1]
                    nc.tensor.matmul(
                        vps[rb][:], lhsT=wap(WVOFF, 128),
                        rhs=xap(xo, 512, W), start=True, stop=False,
                        perf_mode=DR)
                    nc.tensor.matmul(
                        vps[rb][:], lhsT=wap(ZOFF, WVOFF - ZOFF),
                        rhs=xap(xo, 512, W), start=False, stop=True,
                        perf_mode=DR)

                def chain(rb, wci, ws, n, mule=None):
                    mule = mule or nc.vector
                    ee = sb.tile([P, 2, n, L], bf16, tag=f"e{rb}{wci}",
                                 name=f"e{rb}{wci}")
                    nc.scalar.activation(
                        ee[:, 0], scores[(rb, wci)][:],
                        func=mybir.ActivationFunctionType.Exp,
                        scale=1.0 / (SW * SX))
                    mule.tensor_mul(ee[:, 1], ee[:, 0],
                                    vsb[rb][:, ws:ws + n, :])
                    out_ap = (osum if rb == 0
                              else osum1)[:, :, ws:ws + n]
                    with nc.allow_low_precision(
                            reason="32-term bf16 sums; rel tol 2e-2"):
                        nc.vector.tensor_reduce(
                            out=out_ap, in_=ee[:],
                            axis=mybir.AxisListType.X,
                            op=mybir.AluOpType.add)

                conv_group(0, 0, WCS0)
                conv_group(0, 1, WCS0)
                vproj(0)
                nc.scalar.activation(vsb[0][:], vps[0][:],
                                     func=mybir.ActivationFunctionType.Copy,
                                     scale=1.0 / (SWV * SX))
                # bank-major p2 passes so bank0's scores (and its softmax
                # chain, which seeds the serial DVE tail) finish as soon as
                # chunk 3 lands.
                for wci, (ws, n) in enumerate(WCS0):
                    conv_pass(0, wci, ws, n, 2)
                    chain(0, wci, ws, n)
                    if wci == 0:
                        vproj(1)
                nc.scalar.activation(vsb[1][:], vps[1][:],
                                     func=mybir.ActivationFunctionType.Copy,
                                     scale=1.0 / (SWV * SX))
                nc.sync.dma_start(out=o_d[:], in_=osum[:])
                for wci, (ws, n) in enumerate(WCS1):
                    conv_bank(1, wci, ws, n)
                    chain(1, wci, ws, n)
                nc.sync.dma_start(
                    out=o2_d[:],
                    in_=osum1[:])

        nc.compile()
        self.nc = nc


def _get_plan():
    global _PLAN
    if _PLAN is None:
        _PLAN = _Plan()
    return _PLAN


def _prep_in_maps(x, W1, W2):
    fp8 = _fp8()

    W1k = W1[C:2 * C, :, 0, 0].astype(np.float64)
    W2eff = np.einsum("okyx,kc->ocyx", W2.astype(np.float64),
                      W1k).astype(np.float32)
    W1v = W1[2 * C:3 * C, :, 0, 0].astype(np.float32)

    wtiles = np.zeros((3, 5, P, P), np.float32)
    for p in range(3):
        for dx in range(5):
            for s in range(2):
                for rh in range(2):
                    dyi = 2 * p + s - rh
                    if 0 <= dyi <= 4:
                        wtiles[p, dx,
                               64 * s:64 * s + 64,
                               64 * rh:64 * rh + 64] = W2eff[:, :, dyi, dx].T
    wv = np.zeros((P, P), np.float32)
    wv[:C, :C] = W1v.T
    wv[C:, C:] = W1v.T

    def hilo(a, scale):
        hi = (a * scale).astype(fp8)
        lo = (a * scale - hi.astype(np.float32)).astype(fp8)
        return hi, lo

    whi, wlo = hilo(wtiles, SW)          # [3,5,P,P]
    wvhi, wvlo = hilo(wv, SWV)           # [P,P]

    xp = np.zeros((B, C, H + 4, W, L), np.float32)
    xp[:, :, 2:2 + H] = x
    in_maps = []
    for m in range(NCORES):
        bm, q = m // 4, m % 4
        rows = xp[bm, :, 4 * q:4 * q + 8]
        tiles = rows.reshape(C, NPAIR, 2, W, L).transpose(2, 0, 1, 3, 4)
        tiles = tiles.reshape(P, NPAIR, W * L)
        thi, tlo = hilo(tiles, SX)

        blob = np.zeros((P, BLOB), np.float32)
        for p in range(3):
            for dx in range(5):
                off = WT[p] + 256 * TAPIDX[dx]
                blob[:, off:off + 128] = whi[p, dx]
                blob[:, off + 128:off + 256] = wlo[p, dx]
        blob[:, WVOFF:WVOFF + 128] = wvhi
        blob[:, WVOFF + 128:WVOFF + 256] = wvlo
        bq = blob.astype(fp8)
        for t in range(NPAIR):
            bq[:, XP[t]:XP[t] + W * L] = tlo[:, t]
            bq[:, XP[t] + W * L:XP[t] + 2 * W * L] = thi[:, t]
        im = {f"d{k}": np.ascontiguousarray(bq[:, st:st + ln])
              for k, (st, ln) in enumerate(CHUNKS)}
        in_maps.append(im)
    return in_maps


def kernel(x, W1, b1, W2, b2):
    from concourse.bass_utils import run_bass_kernel_spmd

    x = np.asarray(x, dtype=np.float32)
    W1 = np.asarray(W1, dtype=np.float32)
    b1 = np.asarray(b1, dtype=np.float32)
    W2 = np.asarray(W2, dtype=np.float32)

    plan = _get_plan()
    in_maps = _prep_in_maps(x, W1, W2)
    res = run_bass_kernel_spmd(plan.nc, in_maps, core_ids=list(range(NCORES)))

    b1v = b1[2 * C:3 * C].astype(np.float32)
    out = np.empty((B, C, H, W, L), np.float32)
    for m in range(NCORES):
        bm, q = m // 4, m % 4
        o0 = np.asarray(res.results[m]["o"], dtype=np.float32)
        o1 = np.asarray(res.results[m]["o2"],
                        dtype=np.float32).reshape(P, 2, W)
        o = np.stack([o0.reshape(P, 2, W), o1], axis=1)
        o = o.reshape(2, C, 2, 2, W)
        val = o[:, :, :, 1] / o[:, :, :, 0]
        val = val + b1v[None, :, None, None]
        val = val.transpose(1, 2, 0, 3).reshape(C, 4, W)
        out[bm, :, 4 * q:4 * q + 4, :, :] = val[..., None]
    return out
